# revision 19
# baseline (speedup 1.0000x reference)
"""GAT layer (nn_GAT_21930103013469) on 8 trn2 NeuronCores.

Reference (per batch b):
    Wh  = h @ W                                   [N, F]
    s1  = Wh @ a1,  s2 = Wh @ a2                  [N]
    e   = leakyrelu(s1[:,None] + s2[None,:], 0.2) [N, N]
    att = softmax(where(adj>0, e, -9e15), axis=1)   (normalized over rows i)
    out = elu(att @ Wh)

Data parallel over B=16 (2 batches per core). Per batch the attention
matrix is computed TRANSPOSED (PT[j, i]): the softmax reduction (over i)
is a free-dim reduction fused into the ACT Exp pass (accum_out), and the
output matmul out^T[o, i] = sum_j V[j, o] * PT[j, i] contracts j on
partitions. adj is loaded raw (int32, full 128-row blocks on the sync
HWDGE queue), cast to fp8 {0,1} on DVE, and each 128x128 block is
multiplied with a scaled fp8 identity (out = adj_blk^T @ cI = c*adjT),
landing the mask pre-scaled in PSUM; a rank-1 f32r matmul adds s1[i]
along the free dim and s2[j] rides in as the per-partition ACT bias.

This version software-pipelines the two batches: the PV (att @ Wh)
matmuls of batch b-1 and the prep (hT/Wh/s) work of batch b+1 are
interleaved instruction-by-instruction into the j-sweep windows of
batch b, so the tensor engine never idles behind the ACT/DVE
elementwise chain and stays at its ramped clock. PV accumulates one
i-half (4 PSUM banks) per sweep window, trailing the V[jt] production
by TRAIL units. The elu epilogue runs fp16 and the host transposes and
upcasts.
"""
import sys

sys.path.insert(0, "/opt/trn_rl_repo")

import contextlib

import numpy as np
import ml_dtypes

import concourse.bacc as bacc
import concourse.tile as tile
from concourse import mybir
from concourse.bass_utils import run_bass_kernel_spmd

B, N, F = 16, 2048, 256
NCORES = 8
BPC = B // NCORES          # batches per core
NT = N // 128              # 16 i/j tiles
FT = F // 128              # 2 fin/fout tiles
ALPHA = 0.2
SHIFT = 10.0               # global exponent shift: PT = exp(u - SHIFT)
VSCALE = 8.0               # with e^SHIFT from 1/Z', keeps V in fp16 range
TRAIL = 2                  # pv trails V-production by this many jt units

f32, f32r, bf16, fp8, i32 = (
    mybir.dt.float32, mybir.dt.float32r, mybir.dt.bfloat16,
    mybir.dt.float8e4, mybir.dt.int32,
)
f16 = mybir.dt.float16
AF = mybir.ActivationFunctionType
OP = mybir.AluOpType


def route_dve(jt, hi):
    # which (jt, hi) units use the DVE leaky-relu route (vs ACT Prelu);
    # ~14/32, interleaved so neither engine idles in runs
    return (jt * 2 + hi) % 16 < 7


def build_nc(debug=False):
    nc = bacc.Bacc("TRN2", target_bir_lowering=False)
    h_d = nc.dram_tensor("h", [BPC, N, F], f32, kind="ExternalInput")
    adj_d = nc.dram_tensor("adj", [BPC, N, N], i32, kind="ExternalInput")
    W_d = nc.dram_tensor("w", [BPC, F, F], f32, kind="ExternalInput")
    a_d = nc.dram_tensor("a", [BPC, 2 * F, 1], f32, kind="ExternalInput")
    identB_d = nc.dram_tensor("identb", [128, 512], fp8, kind="ExternalInput")
    identC_d = nc.dram_tensor("identc", [128, 512], fp8, kind="ExternalInput")
    ident1_d = nc.dram_tensor("ident1", [128, 128], f32, kind="ExternalInput")
    out_d = nc.dram_tensor("out", [BPC, F, N], f16, kind="ExternalOutput")

    with contextlib.ExitStack() as st:
        tc = st.enter_context(tile.TileContext(nc))
        const = st.enter_context(tc.tile_pool(name="const", bufs=1))
        hin = st.enter_context(tc.tile_pool(name="hin", bufs=3))
        htp = st.enter_context(tc.tile_pool(name="ht", bufs=1))
        wa = st.enter_context(tc.tile_pool(name="wa", bufs=2))
        scp = st.enter_context(tc.tile_pool(name="sc", bufs=2))
        mnatp = st.enter_context(tc.tile_pool(name="mnat", bufs=8))
        arawp = st.enter_context(tc.tile_pool(name="araw", bufs=3))
        ptp = st.enter_context(tc.tile_pool(name="pt", bufs=NT + 1))
        vsp = st.enter_context(tc.tile_pool(name="vs", bufs=NT))
        uup = st.enter_context(tc.tile_pool(name="uu", bufs=1))
        rwp = st.enter_context(tc.tile_pool(name="rw", bufs=2))
        zzp = st.enter_context(tc.tile_pool(name="zz", bufs=8))
        epp = st.enter_context(tc.tile_pool(name="ep", bufs=2))
        whp = st.enter_context(tc.tile_pool(name="whs", bufs=2 * NT + 1))
        psS = st.enter_context(tc.tile_pool(name="psS", bufs=2, space="PSUM"))
        psO = st.enter_context(tc.tile_pool(name="psO", bufs=4, space="PSUM"))

        identB = const.tile([128, 2, 256], fp8)
        nc.sync.dma_start(out=identB, in_=identB_d[:, :])
        identC = const.tile([128, 2, 256], fp8)
        nc.sync.dma_start(out=identC, in_=identC_d[:, :])
        ident1 = const.tile([128, 128], f32)
        nc.sync.dma_start(out=ident1, in_=ident1_d[:, :])
        negshift = const.tile([128, 1], f32)
        nc.vector.memset(negshift, -SHIFT)
        ones_f = const.tile([65, 128], f32)
        nc.vector.memset(ones_f, 1.0)
        ones_r = const.tile([65, 128], f32r)
        nc.vector.tensor_copy(ones_r, ones_f)
        ones_f08 = const.tile([65, 128], f32)
        nc.vector.memset(ones_f08, 0.8)
        ones_r08 = const.tile([65, 128], f32r)
        nc.vector.tensor_copy(ones_r08, ones_f08)

        state = {}

        # ---------- DMA emission (sync queue: adj; scalar queue: h/W/a)

        def kick_adj(b):
            raws = state.setdefault((b, "raw"), {})
            for it in range(NT):
                raw = arawp.tile([128, N], i32, tag="araw",
                                 name=f"araw_{b}_{it}")
                nc.sync.dma_start(
                    out=raw, in_=adj_d[b, it * 128:(it + 1) * 128, :])
                raws[it] = raw

        def kick_h(b, q):
            hss = state.setdefault((b, "h"), {})
            eng = nc.scalar if q == 0 else nc.sync
            for it in range(NT):
                hs = hin.tile([128, F], f32, tag="h", name=f"h_{b}_{it}")
                eng.dma_start(out=hs, in_=h_d[b, it * 128:(it + 1) * 128, :])
                hss[it] = hs

        def kick_wa(b):
            Wsb = wa.tile([128, FT, F], f32, tag="w", name=f"w_{b}")
            nc.gpsimd.dma_start(
                out=Wsb, in_=W_d[b].rearrange("(kt p) o -> p kt o", p=128))
            asb = wa.tile([128, 2, 2], f32, tag="a", name=f"a_{b}")
            for k in range(2):
                for ot in range(2):
                    lo = k * 256 + ot * 128
                    nc.gpsimd.dma_start(out=asb[:, ot, k:k + 1],
                                        in_=a_d[b, lo:lo + 128, :])
            state[b, "wa"] = (Wsb, asb)

        # ---------- adj cast (DVE): raw int32 -> fp8 {0,1}, two halves

        def cast_adj(b, it, half):
            # mask tiles live as it-PAIRS [128, 2, N] so the DoubleRow
            # mask matmul can take both k-tiles in one stationary AP
            raws = state[b, "raw"]
            mnat = state.setdefault((b, "mnat"), {})
            pr = it // 2
            if it % 2 == 0 and half == 0:
                m = mnatp.tile([128, 2, N], fp8, tag="mnat",
                               name=f"mnat_{b}_{pr}")
                mnat[pr] = m
            m = mnat[pr]
            sl = slice(half * (N // 2), (half + 1) * (N // 2))
            head = (b == 0 and it < 8)
            eng = (nc.gpsimd if (it % 2 == 1 and half == 1 and not head)
                   else nc.vector)
            eng.tensor_copy(m[:, it % 2, sl], raws[it][:, sl])

        # ---------- prep: hT, W->f32r, c, s1 rows, sT, biases, Wh

        def prep_hT(b, g):
            # one group: transpose 4 h strips into hT[:, :, g*512:(g+1)*512]
            if g == 0:
                state[b, "hT"] = htp.tile([128, FT, N], f16, tag="ht",
                                          name=f"ht_{b}")
            hT = state[b, "hT"]
            hss = state[b, "h"]
            phts = [psO.tile([128, 512], f32, tag="O",
                             name=f"pht_{b}_{g}_{ft}") for ft in range(FT)]
            # q outer so each h tile is fully released before the next
            # ring slot is needed (hin ring < 4)
            for q in range(4):
                for ft in range(FT):
                    nc.tensor.transpose(
                        phts[ft][:, q * 128:(q + 1) * 128],
                        hss[4 * g + q][:, ft * 128:(ft + 1) * 128], ident1)
            for ft in range(FT):
                nc.vector.tensor_copy(hT[:, ft, g * 512:(g + 1) * 512],
                                      phts[ft])

        def prep_scalars(b):
            # WT, c = W @ a, s1 rows, sT (s2 per-partition), biases
            Wsb, asb = state[b, "wa"]
            hT = state[b, "hT"]
            Wr = wa.tile([128, FT, F], f16, tag="wr", name=f"wr_{b}")
            nc.vector.tensor_copy(Wr, Wsb)
            state[b, "Wr"] = Wr

            WT = wa.tile([128, FT, F], f32, tag="wt", name=f"wt_{b}")
            for ot in range(FT):
                pwt = psO.tile([128, 512], f32, tag="O", name=f"pwt_{b}_{ot}")
                for kt in range(FT):
                    nc.tensor.transpose(
                        pwt[:, kt * 128:(kt + 1) * 128],
                        Wsb[:, kt, ot * 128:(ot + 1) * 128], ident1)
                nc.vector.tensor_copy(WT[:, ot, :F], pwt[:, :F])

            csb = scp.tile([128, FT, 2], f16, tag="c", name=f"c_{b}")
            for ft in range(FT):
                pc = psO.tile([128, 512], f32, tag="O", name=f"pc_{b}_{ft}")
                for ot in range(FT):
                    nc.tensor.matmul(
                        pc[:, 0:2], WT[:, ot, ft * 128:(ft + 1) * 128],
                        asb[:, ot, :], start=(ot == 0), stop=(ot == FT - 1))
                nc.vector.tensor_copy(csb[:, ft, :], pc[:, 0:2])

            # s1 row packed on partition rows 0 and 64 (valid matmul
            # base partitions); 0.8 scale rides in the rank-1 stationary
            s1r = scp.tile([65, 1024], f32r, tag="s1r", bufs=2,
                           name=f"s1r_{b}")
            for ch in range(4):
                sl = slice(ch * 512, (ch + 1) * 512)
                ps = psO.tile([2, 512], f32, tag="O",
                              name=f"ps_{b}_{ch}")
                for ft in range(FT):
                    nc.tensor.matmul(ps, csb[:, ft, :], hT[:, ft, sl],
                                     start=(ft == 0), stop=(ft == FT - 1))
                p0 = 64 * (ch // 2)
                co = 512 * (ch % 2)
                nc.vector.tensor_copy(s1r[p0:p0 + 1, co:co + 512],
                                      ps[0:1, :])
            state[b, "s1"] = s1r

            sT = scp.tile([128, NT, 2], f32, tag="st", name=f"st_{b}")
            for it in range(NT):
                pst = psO.tile([128, 512], f32, tag="O", name=f"pst_{b}_{it}")
                for ft in range(FT):
                    nc.tensor.matmul(
                        pst[:, 0:2], hT[:, ft, it * 128:(it + 1) * 128],
                        csb[:, ft, :], start=(ft == 0), stop=(ft == FT - 1))
                nc.vector.tensor_copy(sT[:, it, :], pst[:, 0:2])

            bias_act = scp.tile([128, NT], f32, tag="ba", name=f"ba_{b}")
            nc.vector.tensor_scalar(
                out=bias_act, in0=sT[:, :, 1], scalar1=1.0, scalar2=-128.0,
                op0=OP.mult, op1=OP.add)
            bias_d1 = scp.tile([128, NT], f32, tag="b1", name=f"b1_{b}")
            nc.vector.tensor_scalar(
                out=bias_d1, in0=sT[:, :, 1], scalar1=0.8, scalar2=-64.0,
                op0=OP.mult, op1=OP.add)
            bias_d2 = scp.tile([128, NT], f32, tag="b2", name=f"b2_{b}")
            nc.vector.tensor_scalar(
                out=bias_d2, in0=sT[:, :, 1], scalar1=0.2,
                scalar2=-16.0 - SHIFT, op0=OP.mult, op1=OP.add)
            state[b, "bias"] = (bias_act, bias_d1, bias_d2)

        def prep_wh(b, jt):
            # Wh[jt] in fp16 from hT and Wr
            hT = state[b, "hT"]
            Wr = state[b, "Wr"]
            whs = state.setdefault((b, "wh"), {})
            pw = psO.tile([128, 512], f32, tag="O", name=f"pw_{b}_{jt}")
            for ft in range(FT):
                nc.tensor.matmul(
                    pw[:, :F], hT[:, ft, jt * 128:(jt + 1) * 128],
                    Wr[:, ft, :], start=(ft == 0), stop=(ft == FT - 1))
            wh = whp.tile([128, F], f16, tag="wh", name=f"wh_{b}_{jt}")
            if jt % 2 == 0:
                nc.scalar.activation(out=wh, in_=pw[:, :F], func=AF.Copy,
                                     bias=0.0, scale=1.0)
            else:
                nc.vector.tensor_copy(wh, pw[:, :F])
            whs[jt] = wh

        # ---------- j-sweep unit: S assembly (PE) + lrelu/exp -> PT, z

        def unit(b, jt, hi):
            mnat = state[b, "mnat"]
            s1r = state[b, "s1"]
            bias_act, bias_d1, bias_d2 = state[b, "bias"]
            pts = state.setdefault((b, "pt"), {})
            if hi == 0:
                pts[jt] = ptp.tile([128, N], f16, tag="pt",
                                   name=f"pt_{b}_{jt}")
            pt = pts[jt]
            dve = route_dve(jt, hi)
            ident = identC if dve else identB
            ones = ones_r08 if dve else ones_r
            S = psS.tile([128, 1024], f32, tag="S", name=f"S_{b}_{jt}_{hi}")
            for pr in range(4):
                nc.tensor.matmul(
                    S[:, pr * 256:(pr + 1) * 256],
                    mnat[hi * 4 + pr][:, :, jt * 128:(jt + 1) * 128],
                    ident, start=(pr % 2 == 0), stop=False,
                    perf_mode=mybir.MatmulPerfMode.DoubleRow)
            for c2 in range(2):
                p0 = 64 * hi
                co = 512 * c2
                nc.tensor.matmul(
                    S[:, c2 * 512:(c2 + 1) * 512], ones[p0:p0 + 1, :],
                    s1r[p0:p0 + 1, co:co + 512], start=False, stop=True)
            z = zzp.tile([128, 1], f32, tag="z", bufs=40,
                         name=f"z_{b}_{jt}_{hi}")
            if dve:
                r = rwp.tile([128, 1024], f32, tag="rw", bufs=4,
                             name=f"r_{b}_{jt}_{hi}")
                nc.vector.tensor_scalar(
                    out=r, in0=S, scalar1=bias_d1[:, jt:jt + 1],
                    scalar2=0.0, op0=OP.add, op1=OP.max)
                w = rwp.tile([128, 1024], f32, tag="rw", bufs=4,
                             name=f"w_{b}_{jt}_{hi}")
                nc.vector.scalar_tensor_tensor(
                    out=w, in0=S, scalar=0.25, in1=r,
                    op0=OP.mult, op1=OP.add)
                nc.scalar.activation(
                    out=pt[:, hi * 1024:(hi + 1) * 1024], in_=w,
                    func=AF.Exp, bias=bias_d2[:, jt:jt + 1],
                    scale=1.0, accum_out=z)
            else:
                u = uup.tile([128, 1024], f16, tag="u",
                             name=f"u_{b}_{jt}_{hi}")
                nc.scalar.activation(
                    out=u, in_=S, func=AF.Prelu,
                    bias=bias_act[:, jt:jt + 1], scale=1.0,
                    alpha=ALPHA)
                nc.scalar.activation(
                    out=pt[:, hi * 1024:(hi + 1) * 1024], in_=u,
                    func=AF.Exp, bias=negshift, scale=1.0,
                    accum_out=z)
            state[b, "z", jt, hi] = z

        def v_finalize(b, jt):
            z0 = state[b, "z", jt, 0]
            z1 = state[b, "z", jt, 1]
            zs = zzp.tile([128, 1], f32, tag="zs", name=f"zs_{b}_{jt}")
            nc.vector.tensor_add(zs, z0, z1)
            zr = zzp.tile([128, 1], f32, tag="zr", name=f"zr_{b}_{jt}")
            nc.vector.reciprocal(zr, zs)
            zrv = zzp.tile([128, 1], f32, tag="zrv", name=f"zrv_{b}_{jt}")
            nc.vector.tensor_scalar_mul(zrv, zr, VSCALE)
            v = vsp.tile([128, F], f16, tag="v", name=f"v_{b}_{jt}")
            nc.vector.tensor_scalar_mul(v, state[b, "wh"][jt], zrv)
            state.setdefault((b, "v"), {})[jt] = v

        # ---------- PV: out^T[o, i-half] += V[jt]^T-contraction, trailing

        def pv_step(b, hi, jt, c2s=(0, 1)):
            pts = state[b, "pt"]
            vs = state[b, "v"]
            if jt == 0:
                pv = state.setdefault((b, "pvO", hi), {})
                for c2 in c2s:
                    for ot in range(FT):
                        pv[ot * 2 + c2] = psO.tile(
                            [128, 512], f32, tag="O",
                            name=f"O_{b}_{hi}_{ot}_{c2}")
            Os = state[b, "pvO", hi]
            for ot in range(FT):
                for c2 in c2s:
                    lo = hi * 1024 + c2 * 512
                    nc.tensor.matmul(
                        Os[ot * 2 + c2],
                        vs[jt][:, ot * 128:(ot + 1) * 128],
                        pts[jt][:, lo:lo + 512],
                        start=(jt == 0), stop=(jt == NT - 1))

        def elu_store(b, hi, c2s=(0, 1)):
            # elu(x) = relu(x) + exp(min(x,0)) - 1;  x = O/VSCALE
            Os = state[b, "pvO", hi]
            for ot in range(FT):
                for c2 in c2s:
                    O = Os[ot * 2 + c2]
                    ch = hi * 2 + c2
                    r = epp.tile([128, 512], f16, tag="er",
                                 name=f"er_{b}_{hi}_{ot}_{c2}")
                    nc.scalar.activation(out=r, in_=O, func=AF.Relu,
                                         bias=0.0, scale=1.0 / VSCALE)
                    mn = epp.tile([128, 512], f16, tag="em",
                                  name=f"em_{b}_{hi}_{ot}_{c2}")
                    nc.vector.scalar_tensor_tensor(
                        out=mn, in0=O, scalar=1.0 / VSCALE, in1=r,
                        op0=OP.mult, op1=OP.subtract)
                    nc.scalar.activation(out=mn, in_=mn, func=AF.Exp,
                                         bias=0.0, scale=1.0)
                    o_sb = epp.tile([128, 512], f16, tag="eo",
                                    name=f"eo_{b}_{hi}_{ot}_{c2}")
                    nc.vector.scalar_tensor_tensor(
                        out=o_sb, in0=mn, scalar=-1.0, in1=r,
                        op0=OP.add, op1=OP.add)
                    nc.gpsimd.dma_start(
                        out=out_d[b, ot * 128:(ot + 1) * 128,
                                  ch * 512:(ch + 1) * 512],
                        in_=o_sb)

        # ---------- emission schedule (BPC == 2) ----------------------

        # DMA kicks: adj(0) then h(1) then adj(1) on sync (ring-gated);
        # h(0)/W/a on scalar (all early, before ACT compute starts).
        kick_adj(0)
        kick_h(0, q=0)
        kick_wa(0)
        kick_wa(1)

        # prep(0): runs while adj(0) panel 0 streams in; casts
        # interleave with prep so the araw ring never gates the DMA
        cast_adj(0, 0, 0)
        cast_adj(0, 0, 1)
        for g in range(4):
            prep_hT(0, g)
            cast_adj(0, g + 1, 0)
            cast_adj(0, g + 1, 1)
        prep_scalars(0)
        for it in (5, 6):
            cast_adj(0, it, 0)
            cast_adj(0, it, 1)
        for jt in range(NT):
            prep_wh(0, jt)
            if jt in (0, 2):
                cast_adj(0, 7, jt // 2)

        kick_h(1, q=1)
        kick_adj(1)

        # sweep(0, h0): units + prep(1) interleaved; casts of panel1(0)
        for jt in range(NT):
            unit(0, jt, 0)
            if jt < 4:
                prep_hT(1, jt)
            elif jt == 4:
                prep_scalars(1)
            elif 5 <= jt < 13:
                it = 8 + (jt - 5)
                cast_adj(0, it, 0)
                cast_adj(0, it, 1)
                prep_wh(1, 2 * (jt - 5))
                prep_wh(1, 2 * (jt - 5) + 1)
        cast_adj(0, 15, 0)
        cast_adj(0, 15, 1)

        # sweep(0, h1): units + v_finalize + pv(0, hi=0) trailing in
        # bursts of 4 jt (16 matmuls) so the PE clock can ramp
        for jt in range(NT):
            unit(0, jt, 1)
            v_finalize(0, jt)
            if jt in (5, 9, 13):
                for j2 in range(jt - 5, jt - 1):
                    pv_step(0, 0, j2)
            if jt % 2 == 0:
                it = jt // 2
                cast_adj(1, it, 0)
                cast_adj(1, it, 1)
        for jt in range(NT - 4, NT):
            pv_step(0, 0, jt)
        elu_store(0, 0)

        # sweep(1, h0): units + pv(0, hi=1) in bursts of 4 jt
        for jt in range(NT):
            unit(1, jt, 0)
            if jt % 4 == 3:
                for j2 in range(jt - 3, jt + 1):
                    pv_step(0, 1, j2)
            if jt % 2 == 0:
                it = 8 + jt // 2
                cast_adj(1, it, 0)
                cast_adj(1, it, 1)
        elu_store(0, 1)

        # sweep(1, h1): units + v_finalize + pv(1, hi=0) in bursts
        for jt in range(NT):
            unit(1, jt, 1)
            v_finalize(1, jt)
            if jt in (5, 9, 13):
                for j2 in range(jt - 5, jt - 1):
                    pv_step(1, 0, j2)
        for jt in range(NT - 4, NT):
            pv_step(1, 0, jt)
        elu_store(1, 0)

        # tail: pv(1, hi=1) split in half-pairs so elu overlaps PE
        for jt in range(NT):
            pv_step(1, 1, jt, c2s=(0,))
        elu_store(1, 1, c2s=(0,))
        for jt in range(NT):
            pv_step(1, 1, jt, c2s=(1,))
        elu_store(1, 1, c2s=(1,))

    nc.compile()
    return nc


_NC_CACHE = {}


def _get_nc():
    if "nc" not in _NC_CACHE:
        _NC_CACHE["nc"] = build_nc()
    return _NC_CACHE["nc"]


def build_in_maps(h, adj, W, a):
    eye = np.eye(128, dtype=np.float32)
    zero = np.zeros((128, 128), dtype=np.float32)
    bd = np.concatenate([eye, zero, zero, eye], axis=1)  # [128, 512]
    identB = (bd * 128.0).astype(ml_dtypes.float8_e4m3)
    identC = (bd * 64.0).astype(ml_dtypes.float8_e4m3)
    ident1 = np.eye(128, dtype=np.float32)
    in_maps = []
    for c in range(NCORES):
        sl = slice(c * BPC, (c + 1) * BPC)
        in_maps.append({
            "h": np.ascontiguousarray(h[sl]),
            "adj": np.ascontiguousarray(adj[sl]),
            "w": np.ascontiguousarray(W[sl]),
            "a": np.ascontiguousarray(a[sl]),
            "identb": identB,
            "identc": identC,
            "ident1": ident1,
        })
    return in_maps


def kernel(h, adj, W, a):
    nc = _get_nc()
    res = run_bass_kernel_spmd(nc, build_in_maps(h, adj, W, a),
                               list(range(NCORES)))
    outs = [np.asarray(r["out"]) for r in res.results]   # each [BPC, F, N]
    full = np.concatenate(outs, axis=0)                  # [B, F, N]
    return np.ascontiguousarray(
        full.transpose(0, 2, 1)).astype(np.float32)
